# revision 9
# baseline (speedup 1.0000x reference)
"""MultiHeadCrossAttention Trainium2 kernel (8 NeuronCores, SPMD).

Sharding: core c -> (n = c // 2, g = c % 2). Each core handles one query
batch n and half the heads (8 of 16, embed slice g*512:(g+1)*512).

Host side: transpose queries/keys/values into [dim, tokens] layout (the
TensorEngine contracts along the partition dim, so both matmul operands
need the contraction dim on partitions), compact keys/values along KLEN
by the per-n mask (~50% survive), pad to KC = 128*T, cast to bf16.
The unnormalized AV outputs and softmax denominators come back per core;
the host divides while assembling/transposing the full output.

Device side per core (all matmuls bf16, fp32 PSUM accumulation):
  - qT/kT projections in transposed layout (lhsT = W chunk, rhs = xT);
    kT lands in per-head zero-padded slots (kTz) so the energy matmuls
    run with K=128 - full PE-array activity keeps the HAM clock at
    2.4 GHz (K=64 matmuls measurably re-throttle the PE to 1.2 GHz).
  - v projection in [k, emb] layout (lhsT = vT k-tile, rhs = W chunk).
  - energyT[k, q] = kTz.T @ qT per head, one PSUM bank per (head, k-tile).
  - exp on ScalarE (scale=1/8) PSUM->SBUF bf16. This is the kernel
    bottleneck: softmax exp runs at 1 elem/cycle/lane at 1.2 GHz and only
    ScalarE can do it, so the whole kernel is paced by the ACTIVATE
    stream. Items are (head, q-chunk, group of 3 k-tiles): one FD=1536
    ACTIVATE per item (96 total) amortizes the ~300-cycle per-ACTIVATE
    overhead that an FD=1024 split pays 160 times.
  - AV with lhsT = [v_h | valid-indicator | filler] (M=128) accumulated
    over k-tiles into one PSUM bank per (head, q-chunk): row 64 of the
    accumulator is the softmax denominator for free. Padded k rows have
    v=0 and indicator=0 so they contribute nothing anywhere.
  - PSUM budget (8 banks): energy 2 bufs x 3 banks + projections 1 +
    AV accumulator 1.
  - software pipeline: energy of item i+1 runs on the PE while exp of
    item i streams on ScalarE, kept alive across q-chunk AND head-pair
    boundaries; projection steps (incl. c=0's own) are injected into the
    item stream with data-deadlines so the first exp fires as soon as
    the first weight/q/k DMA chunks land (~10us instead of ~34us);
    inputs stream in column-chunks ordered by first use; junk matmuls
    during the initial DMA window pre-warm the PE clock gate; a dummy
    ACTIVATE preloads the exp table set (~2.7us) off the critical path.
"""

import math
import sys
from contextlib import ExitStack

import numpy as np

for _p in ("/opt/trn_rl_repo",):
    if _p not in sys.path:
        sys.path.insert(0, _p)

import ml_dtypes

import concourse.bass as bass  # noqa: F401  (import registers lowering deps)
import concourse.tile as tile
from concourse import bacc, mybir
from concourse.bass_utils import run_bass_kernel_spmd

BF16 = ml_dtypes.bfloat16

N, QLEN, KLEN = 4, 2048, 2048
QDIM = KVDIM = 512
EMBED, HEADS = 1024, 16
HEAD_DIM = 64
N_CORES = 8
QCH = 512  # q-chunk width (one PSUM bank of fp32)
SCALE = 1.0 / math.sqrt(HEAD_DIM)

_cache: dict = {}
last_exec_time_ns = None
last_results = None


def _build(T: int, ql: int = QLEN):
    """Build the per-core Bass program for KC = 128*T compacted kv tokens."""
    KC = 128 * T
    dt = mybir.dt
    nc = bacc.Bacc("TRN2", target_bir_lowering=False, debug=False)

    qT_d = nc.dram_tensor("qt", [QDIM, ql], dt.bfloat16, kind="ExternalInput").ap()
    kT_d = nc.dram_tensor("kt", [KVDIM, KC], dt.bfloat16, kind="ExternalInput").ap()
    vT_d = nc.dram_tensor("vt", [KVDIM, KC], dt.bfloat16, kind="ExternalInput").ap()
    wq_d = nc.dram_tensor("wq", [QDIM, 512], dt.bfloat16, kind="ExternalInput").ap()
    wk_d = nc.dram_tensor("wk", [KVDIM, 512], dt.bfloat16, kind="ExternalInput").ap()
    wv_d = nc.dram_tensor("wv", [KVDIM, 512], dt.bfloat16, kind="ExternalInput").ap()
    # per-row validity indicator (1.0 real kv token, 0.0 pad), [128, T]
    vind_d = nc.dram_tensor("vind", [128, T], dt.float32, kind="ExternalInput").ap()
    # rows (c*2+h)*65 .. +64: unnormalized AV.T ; row +64: denominator
    out_d = nc.dram_tensor("out", [520, ql], dt.float32, kind="ExternalOutput").ap()

    # [p, j, col] views: one DMA instruction loads all 4 j-tiles of a
    # column chunk (DMA issue cost is per instruction, ~0.6us each on the
    # issuing queue - instruction count, not bytes, gates the startup)
    qT_r = qT_d.rearrange("(j p) c -> p j c", j=4, p=128)
    kT_r = kT_d.rearrange("(j p) c -> p j c", j=4, p=128)
    vT_r = vT_d.rearrange("(j p) c -> p j c", j=4, p=128)
    w_r = {"wq": wq_d.rearrange("(j p) c -> p j c", j=4, p=128),
           "wk": wk_d.rearrange("(j p) c -> p j c", j=4, p=128),
           "wv": wv_d.rearrange("(j p) c -> p j c", j=4, p=128)}

    NQ = ql // QCH
    kcols = [(s, min(512, KC - s)) for s in range(0, KC, 512)]
    # k-tile groups of 3 for batched exp (psE tile = 3 banks, FD=1536)
    groups = [tuple(range(t, min(t + 3, T))) for t in range(0, T, 3)]
    GW = 512 * min(3, T)

    with tile.TileContext(nc) as tc:
        with ExitStack() as ctx:
            persist = ctx.enter_context(tc.tile_pool(name="persist", bufs=1))

            qTin = persist.tile([128, 4, ql], dt.bfloat16, tag="qTin", name="qTin")
            kTin = persist.tile([128, 4, KC], dt.bfloat16, tag="kTin", name="kTin")
            vTin = persist.tile([128, 4, KC], dt.bfloat16, tag="vTin", name="vTin")
            wsb = {nm: persist.tile([128, 4, 512], dt.bfloat16, tag=nm, name=nm)
                   for nm in ("wq", "wk", "wv")}
            qT = [persist.tile([128, ql], dt.bfloat16, tag=f"qT{c}", name=f"qT{c}") for c in range(4)]
            # kTz[c][:, h, :]: rows h*64..h*64+63 hold head h's kT rows, the
            # other 64 rows stay zero -> energy matmuls run with K=128 (full
            # PE array activity) at the same stream cost.
            kTz = [persist.tile([128, 2, KC], dt.bfloat16, tag=f"kTz{c}", name=f"kTz{c}") for c in range(4)]
            # [v_h (64) | indicator (1) | filler (63)]: M=128 keeps the full
            # array busy; output rows 65-127 are ignored.
            vsb = persist.tile([128, T, 4, 2, 128], dt.bfloat16, tag="v", name="v")
            vind = persist.tile([128, T], dt.float32, tag="vind", name="vind")
            junk = persist.tile([128, 512], dt.bfloat16, tag="junk", name="junk")
            dum = persist.tile([128, 1], dt.float32, tag="dum", name="dum")
            dumo = persist.tile([128, 1], dt.float32, tag="dumo", name="dumo")

            # ---- init: preload the exp table set + early memsets for the
            # tiles the first items touch; the rest are deferred into the
            # projection closures so they don't head-of-line block the DVE.
            nc.vector.memset(dum, 0.0)
            nc.scalar.activation(dumo, dum, mybir.ActivationFunctionType.Exp,
                                 scale=1.0)
            nc.vector.memset(junk, 1.0)
            nc.vector.memset(kTz[0], 0.0)
            nc.vector.memset(vsb[:, 0:min(3, T)], 1.0)

            # ---- input DMA. Critical path (c=0 cols of wq/wk, full wv,
            # first 512 kv cols, first q chunk) on the Sync queue; bulk
            # tails stream in parallel on the GpSimd software-DGE queue.
            c0 = min(512, KC)
            nc.sync.dma_start(wsb["wq"][:, :, 0:128], w_r["wq"][:, :, 0:128])
            nc.sync.dma_start(wsb["wk"][:, :, 0:128], w_r["wk"][:, :, 0:128])
            nc.sync.dma_start(vind, vind_d)
            nc.sync.dma_start(qTin[:, :, 0:512], qT_r[:, :, 0:512])
            nc.sync.dma_start(kTin[:, :, 0:c0], kT_r[:, :, 0:c0])
            nc.sync.dma_start(wsb["wv"], w_r["wv"])
            nc.sync.dma_start(vTin[:, :, 0:c0], vT_r[:, :, 0:c0])
            if KC > 512:
                nc.gpsimd.dma_start(kTin[:, :, 512:KC], kT_r[:, :, 512:KC])
                nc.gpsimd.dma_start(vTin[:, :, 512:KC], vT_r[:, :, 512:KC])
            nc.gpsimd.dma_start(qTin[:, :, 512:ql], qT_r[:, :, 512:ql])
            nc.gpsimd.dma_start(wsb["wq"][:, :, 128:512], w_r["wq"][:, :, 128:512])
            nc.gpsimd.dma_start(wsb["wk"][:, :, 128:512], w_r["wk"][:, :, 128:512])

            with tc.tile_pool(name="psA", bufs=1, space="PSUM") as psA, \
                 tc.tile_pool(name="psE", bufs=2, space="PSUM") as psE, \
                 tc.tile_pool(name="psO", bufs=1, space="PSUM") as psO, \
                 tc.tile_pool(name="sbx", bufs=4) as sbx, \
                 tc.tile_pool(name="sbo", bufs=3) as sbo:

                # PE warm-up during the DMA window (HAM un-throttle)
                ps = psA.tile([128, QCH], dt.float32, tag="pA", name="pA")
                for r in range(6):
                    nc.tensor.matmul(ps, lhsT=junk[:, :128], rhs=junk,
                                     start=(r == 0), stop=(r == 5))

                # ---- projection steps (closures, injected into the
                #      attention item stream) ----
                def qp(c, q0):
                    ps = psA.tile([128, QCH], dt.float32, tag="pA", name="pA")
                    for j in range(4):
                        nc.tensor.matmul(
                            ps,
                            lhsT=wsb["wq"][:, j, c * 128:(c + 1) * 128],
                            rhs=qTin[:, j, q0 * QCH:(q0 + 1) * QCH],
                            start=(j == 0), stop=(j == 3),
                        )
                    nc.vector.tensor_copy(qT[c][:, q0 * QCH:(q0 + 1) * QCH], ps)

                def kp(c, ki):
                    s, w = kcols[ki]
                    if ki == 0 and c > 0:
                        nc.vector.memset(kTz[c], 0.0)
                    ps = psA.tile([128, QCH], dt.float32, tag="pA", name="pA")
                    for j in range(4):
                        nc.tensor.matmul(
                            ps[:, :w],
                            lhsT=wsb["wk"][:, j, c * 128:(c + 1) * 128],
                            rhs=kTin[:, j, s:s + w],
                            start=(j == 0), stop=(j == 3),
                        )
                    nc.vector.tensor_copy(kTz[c][0:64, 0, s:s + w], ps[0:64, :w])
                    nc.vector.tensor_copy(kTz[c][64:128, 1, s:s + w], ps[64:128, :w])

                def vp(t):
                    # one N=512 matmul projects k-tile t for ALL 4 c-chunks
                    if t >= 3:
                        nc.vector.memset(vsb[:, t], 1.0)
                    ps = psA.tile([128, QCH], dt.float32, tag="pA", name="pA")
                    for j in range(4):
                        nc.tensor.matmul(
                            ps,
                            lhsT=vTin[:, j, t * 128:(t + 1) * 128],
                            rhs=wsb["wv"][:, j, :],
                            start=(j == 0), stop=(j == 3),
                        )
                    for c in range(4):
                        for h in range(2):
                            nc.vector.tensor_copy(
                                vsb[:, t, c, h, 0:64],
                                ps[:, c * 128 + h * 64:c * 128 + (h + 1) * 64])
                            nc.vector.tensor_copy(vsb[:, t, c, h, 64:65],
                                                  vind[:, t:t + 1])

                def proj_tasks(c):
                    # vproj is done once (all c) during c=0's stream
                    return ([lambda q0=q0: qp(c, q0) for q0 in range(NQ)]
                            + [lambda ki=ki: kp(c, ki) for ki in range(len(kcols))])

                # ---- attention pipeline, items = (h, q0, group) ----
                items = [(h, q0, gi) for h in range(2) for q0 in range(NQ)
                         for gi in range(len(groups))]
                NG = len(groups)

                def emit_energy(c, h, q0, gi):
                    grp = groups[gi]
                    eh = psE.tile([128, GW], dt.float32, tag="e", name="e")
                    for b, t in enumerate(grp):
                        nc.tensor.matmul(
                            eh[:, b * QCH:(b + 1) * QCH],
                            lhsT=kTz[c][:, h, t * 128:(t + 1) * 128],
                            rhs=qT[c][:, q0 * QCH:(q0 + 1) * QCH],
                            start=True, stop=True,
                        )
                    return eh

                def emit_exp_av(c, h, q0, gi, eh, av):
                    grp = groups[gi]
                    gw = len(grp) * QCH
                    ex = sbx.tile([128, GW], dt.bfloat16, tag="x", name="x")
                    nc.scalar.activation(
                        ex[:, :gw], eh[:, :gw],
                        mybir.ActivationFunctionType.Exp,
                        scale=SCALE,
                    )
                    for b, t in enumerate(grp):
                        nc.tensor.matmul(
                            av,
                            lhsT=vsb[:, t, c, h, :],
                            rhs=ex[:, b * QCH:(b + 1) * QCH],
                            start=(t == 0), stop=(t == T - 1),
                        )

                def emit_out(c, h, q0, av):
                    ot = sbo.tile([65, QCH], dt.float32, tag="ot", name="ot")
                    nc.vector.tensor_copy(ot, av[0:65, :])
                    nc.sync.dma_start(
                        out_d[(c * 2 + h) * 65:(c * 2 + h) * 65 + 65,
                              q0 * QCH:(q0 + 1) * QCH], ot)

                # c=0's own projections carry data deadlines (item index by
                # which they must be emitted); later cs' are paced evenly.
                def c0_deadline(kind, arg):
                    if kind == "q":  # needed by E(0, q0, 0)
                        return arg * NG
                    if kind == "k":  # chunk ki first used by E of the group
                        t_lo = kcols[arg][0] // 128   # holding its first tile
                        return min(gi for gi, grp in enumerate(groups)
                                   if t_lo in grp)
                    # "v": tile t needed by AV(group of t), emitted 1 item late
                    g = next(gi for gi, grp in enumerate(groups) if arg in grp)
                    return min(g + 1, len(items) - 1)

                sched: dict = {i: [] for i in range(len(items))}
                for kind, n_args in (("q", NQ), ("k", len(kcols))):
                    for a in range(n_args):
                        fn = {"q": qp, "k": kp}[kind]
                        sched[c0_deadline(kind, a)].append(
                            lambda a=a, fn=fn: fn(0, a))
                for t in range(T):
                    sched[c0_deadline("v", t)].append(lambda t=t: vp(t))

                av_cur = None

                def flush_prev(prev):
                    nonlocal av_cur
                    pc, ph, pq0, pgi, peh = prev
                    if pgi == 0:
                        av_cur = psO.tile([128, QCH], dt.float32, tag="av",
                                          name="av")
                    emit_exp_av(pc, ph, pq0, pgi, peh, av_cur)
                    if pgi == NG - 1:
                        emit_out(pc, ph, pq0, av_cur)

                prev = None
                for c in range(4):
                    tasks = proj_tasks(c + 1) if c < 3 else []
                    n_it = len(items)
                    # c=0: its own projections run in the first half (gated
                    # by input DMA); c=1's wait for the weight-tail DMAs.
                    s0 = n_it // 2 if c == 0 else 0
                    span = n_it - s0
                    for i, (h, q0, gi) in enumerate(items):
                        if c == 0:
                            for fn in sched[i]:
                                fn()
                        # evenly paced projections for c+1
                        if i >= s0:
                            p = i - s0
                            for fn in tasks[(p * len(tasks)) // span:
                                            ((p + 1) * len(tasks)) // span]:
                                fn()
                        eh = emit_energy(c, h, q0, gi)
                        if prev is not None:
                            flush_prev(prev)
                        prev = (c, h, q0, gi, eh)
                flush_prev(prev)

    nc.compile()
    return nc


def _prepare(queries, keys, values, mask):
    """Host-side sharding: transpose, compact kv by mask, validity tiles."""
    m = np.asarray(mask).reshape(N, KLEN) != 0
    idx = [np.nonzero(m[n])[0] for n in range(N)]
    cnts = [len(i) for i in idx]
    T = max(1, (max(cnts) + 127) // 128)
    KC = 128 * T

    kT_full = np.ascontiguousarray(np.asarray(keys, np.float32)[0].T)
    vT_full = np.ascontiguousarray(np.asarray(values, np.float32)[0].T)
    q32 = np.asarray(queries, np.float32)

    qT_n, kT_n, vT_n, vind_n = [], [], [], []
    for n in range(N):
        kt = np.zeros((KVDIM, KC), np.float32)
        vt = np.zeros((KVDIM, KC), np.float32)
        kt[:, :cnts[n]] = kT_full[:, idx[n]]
        vt[:, :cnts[n]] = vT_full[:, idx[n]]
        ind = (np.arange(KC) < cnts[n]).astype(np.float32)
        vind_n.append(np.ascontiguousarray(ind.reshape(T, 128).T))
        kT_n.append(kt.astype(BF16))
        vT_n.append(vt.astype(BF16))
        qT_n.append(np.ascontiguousarray(q32[n].T).astype(BF16))
    return T, qT_n, kT_n, vT_n, vind_n


def kernel(queries, keys, values, mask, Wq, Wk, Wv, _trace=False):
    global last_exec_time_ns, last_results
    T, qT_n, kT_n, vT_n, vind_n = _prepare(queries, keys, values, mask)

    w_g = {}
    for nm, W in (("wq", Wq), ("wk", Wk), ("wv", Wv)):
        W = np.asarray(W, np.float32)
        w_g[nm] = [np.ascontiguousarray(W[:, g * 512:(g + 1) * 512]).astype(BF16)
                   for g in range(2)]

    nc = _cache.get(T)
    if nc is None:
        nc = _cache.setdefault(T, _build(T))

    in_maps = []
    for core in range(N_CORES):
        n, g = core // 2, core % 2
        in_maps.append({
            "qt": qT_n[n], "kt": kT_n[n], "vt": vT_n[n],
            "wq": w_g["wq"][g], "wk": w_g["wk"][g], "wv": w_g["wv"][g],
            "vind": vind_n[n],
        })

    res = run_bass_kernel_spmd(nc, in_maps, core_ids=list(range(N_CORES)),
                               trace=bool(_trace))
    last_exec_time_ns = res.exec_time_ns
    last_results = res

    full = np.empty((N, QLEN, EMBED), np.float32)
    for core in range(N_CORES):
        n, g = core // 2, core % 2
        o = res.results[core]["out"].reshape(8, 65, QLEN)
        vals = o[:, :64, :] / o[:, 64:65, :]          # [8, 64, QLEN]
        full[n, :, g * 512:(g + 1) * 512] = (
            vals.transpose(2, 0, 1).reshape(QLEN, 512)
        )
    return full


# revision 11
# speedup vs baseline: 1.0217x; 1.0217x over previous
"""MultiHeadCrossAttention Trainium2 kernel (8 NeuronCores, SPMD).

Sharding: core c -> (n = c // 2, g = c % 2). Each core handles one query
batch n and half the heads (8 of 16, embed slice g*512:(g+1)*512).

Host side: transpose queries/keys/values into [dim, tokens] layout (the
TensorEngine contracts along the partition dim, so both matmul operands
need the contraction dim on partitions), compact keys/values along KLEN
by the per-n mask (~50% survive), pad to KC = 128*T, cast to bf16.
The unnormalized AV outputs and softmax denominators come back per core;
the host divides while assembling/transposing the full output.

Device side per core (all matmuls bf16, fp32 PSUM accumulation):
  - qT/kT projections in transposed layout (lhsT = W chunk, rhs = xT);
    kT lands in per-head zero-padded slots (kTz) so the energy matmuls
    run with K=128 - full PE-array activity keeps the HAM clock at
    2.4 GHz (K=64 matmuls measurably re-throttle the PE to 1.2 GHz).
  - v projection in [k, emb] layout (lhsT = vT k-tile, rhs = W chunk).
  - energyT[k, q] = kTz.T @ qT per head, one PSUM bank per (head, k-tile).
  - exp on ScalarE (scale=1/8) PSUM->SBUF bf16. This is the kernel
    bottleneck: softmax exp runs at 1 elem/cycle/lane at 1.2 GHz and only
    ScalarE can do it, so the whole kernel is paced by the ACTIVATE
    stream. Items are (head, q-chunk, group of 3 k-tiles): one FD=1536
    ACTIVATE per item (96 total) amortizes the ~300-cycle per-ACTIVATE
    overhead that an FD=1024 split pays 160 times.
  - AV with lhsT = [v_h | valid-indicator | filler] (M=128) accumulated
    over k-tiles into one PSUM bank per (head, q-chunk): row 64 of the
    accumulator is the softmax denominator for free. Padded k rows have
    v=0 and indicator=0 so they contribute nothing anywhere.
  - PSUM budget (8 banks): energy 2 bufs x 3 banks + projections 1 +
    AV accumulator 1.
  - software pipeline: energy of item i+1 runs on the PE while exp of
    item i streams on ScalarE, kept alive across q-chunk AND head-pair
    boundaries; projection steps (incl. c=0's own) are injected into the
    item stream with data-deadlines so the first exp fires as soon as
    the first weight/q/k DMA chunks land (~10us instead of ~34us);
    inputs stream in column-chunks ordered by first use; junk matmuls
    during the initial DMA window pre-warm the PE clock gate; a dummy
    ACTIVATE preloads the exp table set (~2.7us) off the critical path.
"""

import math
import sys
from contextlib import ExitStack

import numpy as np

for _p in ("/opt/trn_rl_repo",):
    if _p not in sys.path:
        sys.path.insert(0, _p)

import ml_dtypes

import concourse.bass as bass  # noqa: F401  (import registers lowering deps)
import concourse.tile as tile
from concourse import bacc, mybir
from concourse.bass_utils import run_bass_kernel_spmd

BF16 = ml_dtypes.bfloat16

N, QLEN, KLEN = 4, 2048, 2048
QDIM = KVDIM = 512
EMBED, HEADS = 1024, 16
HEAD_DIM = 64
N_CORES = 8
QCH = 512  # q-chunk width (one PSUM bank of fp32)
SCALE = 1.0 / math.sqrt(HEAD_DIM)

_cache: dict = {}
last_exec_time_ns = None
last_results = None


def _build(T: int, ql: int = QLEN):
    """Build the per-core Bass program for KC = 128*T compacted kv tokens."""
    KC = 128 * T
    dt = mybir.dt
    nc = bacc.Bacc("TRN2", target_bir_lowering=False, debug=False)

    qT_d = nc.dram_tensor("qt", [QDIM, ql], dt.bfloat16, kind="ExternalInput").ap()
    kT_d = nc.dram_tensor("kt", [KVDIM, KC], dt.bfloat16, kind="ExternalInput").ap()
    vT_d = nc.dram_tensor("vt", [KVDIM, KC], dt.bfloat16, kind="ExternalInput").ap()
    wq_d = nc.dram_tensor("wq", [QDIM, 512], dt.bfloat16, kind="ExternalInput").ap()
    wk_d = nc.dram_tensor("wk", [KVDIM, 512], dt.bfloat16, kind="ExternalInput").ap()
    wv_d = nc.dram_tensor("wv", [KVDIM, 512], dt.bfloat16, kind="ExternalInput").ap()
    # per-row validity indicator (1.0 real kv token, 0.0 pad), [128, T]
    vind_d = nc.dram_tensor("vind", [128, T], dt.float32, kind="ExternalInput").ap()
    # rows (c*2+h)*65 .. +64: unnormalized AV.T ; row +64: denominator
    out_d = nc.dram_tensor("out", [520, ql], dt.float32, kind="ExternalOutput").ap()

    # [p, j, col] views: one DMA instruction loads all 4 j-tiles of a
    # column chunk (DMA issue cost is per instruction, ~0.6us each on the
    # issuing queue - instruction count, not bytes, gates the startup)
    qT_r = qT_d.rearrange("(j p) c -> p j c", j=4, p=128)
    kT_r = kT_d.rearrange("(j p) c -> p j c", j=4, p=128)
    vT_r = vT_d.rearrange("(j p) c -> p j c", j=4, p=128)
    w_r = {"wq": wq_d.rearrange("(j p) c -> p j c", j=4, p=128),
           "wk": wk_d.rearrange("(j p) c -> p j c", j=4, p=128),
           "wv": wv_d.rearrange("(j p) c -> p j c", j=4, p=128)}

    NQ = ql // QCH
    kcols = [(s, min(512, KC - s)) for s in range(0, KC, 512)]
    # k-tile groups of 3 for batched exp (psE tile = 3 banks, FD=1536)
    groups = [tuple(range(t, min(t + 3, T))) for t in range(0, T, 3)]
    GW = 512 * min(3, T)

    with tile.TileContext(nc) as tc:
        with ExitStack() as ctx:
            persist = ctx.enter_context(tc.tile_pool(name="persist", bufs=1))

            qTin = persist.tile([128, 4, ql], dt.bfloat16, tag="qTin", name="qTin")
            kTin = persist.tile([128, 4, KC], dt.bfloat16, tag="kTin", name="kTin")
            vTin = persist.tile([128, 4, KC], dt.bfloat16, tag="vTin", name="vTin")
            wsb = {nm: persist.tile([128, 4, 512], dt.bfloat16, tag=nm, name=nm)
                   for nm in ("wq", "wk", "wv")}
            qT = [persist.tile([128, ql], dt.bfloat16, tag=f"qT{c}", name=f"qT{c}") for c in range(4)]
            # kTz[c][:, h, :]: rows h*64..h*64+63 hold head h's kT rows, the
            # other 64 rows stay zero -> energy matmuls run with K=128 (full
            # PE array activity) at the same stream cost.
            kTz = [persist.tile([128, 2, KC], dt.bfloat16, tag=f"kTz{c}", name=f"kTz{c}") for c in range(4)]
            # [v_h (64) | indicator (1) | filler (63)]: M=128 keeps the full
            # array busy; output rows 65-127 are ignored.
            vsb = persist.tile([128, T, 4, 2, 128], dt.bfloat16, tag="v", name="v")
            vind = persist.tile([128, T], dt.float32, tag="vind", name="vind")
            junk = persist.tile([128, 512], dt.bfloat16, tag="junk", name="junk")
            dum = persist.tile([128, 1], dt.float32, tag="dum", name="dum")
            dumo = persist.tile([128, 1], dt.float32, tag="dumo", name="dumo")

            # ---- init: preload the exp table set + early memsets for the
            # tiles the first items touch; the rest are deferred into the
            # projection closures so they don't head-of-line block the DVE.
            nc.vector.memset(dum, 0.0)
            nc.scalar.activation(dumo, dum, mybir.ActivationFunctionType.Exp,
                                 scale=1.0)
            nc.vector.memset(junk, 1.0)
            nc.vector.memset(kTz[0], 0.0)
            nc.vector.memset(vsb[:, 0:min(3, T)], 1.0)

            # ---- input DMA, whole [128, X] row-contiguous tiles (DMA
            # transfers are descriptor-rate bound: column-chunked loads
            # have 4x the descriptors and move SLOWER). Ordered by first
            # use; per-j arrival lets the j-accumulation MMs start early.
            for j in range(4):
                nc.sync.dma_start(wsb["wq"][:, j], w_r["wq"][:, j])
            for j in range(4):
                nc.sync.dma_start(wsb["wk"][:, j], w_r["wk"][:, j])
            nc.sync.dma_start(vind, vind_d)
            for j in range(4):
                nc.sync.dma_start(qTin[:, j], qT_r[:, j])
            for j in range(4):
                nc.sync.dma_start(kTin[:, j], kT_r[:, j])
            for j in range(4):
                nc.sync.dma_start(wsb["wv"][:, j], w_r["wv"][:, j])
            for j in range(4):
                nc.sync.dma_start(vTin[:, j], vT_r[:, j])

            with tc.tile_pool(name="psA", bufs=1, space="PSUM") as psA, \
                 tc.tile_pool(name="psE", bufs=2, space="PSUM") as psE, \
                 tc.tile_pool(name="psO", bufs=1, space="PSUM") as psO, \
                 tc.tile_pool(name="sbx", bufs=4) as sbx, \
                 tc.tile_pool(name="sbo", bufs=3) as sbo:

                # PE warm-up during the DMA window (HAM un-throttle)
                ps = psA.tile([128, QCH], dt.float32, tag="pA", name="pA")
                for r in range(6):
                    nc.tensor.matmul(ps, lhsT=junk[:, :128], rhs=junk,
                                     start=(r == 0), stop=(r == 5))

                # ---- projection steps (closures, injected into the
                #      attention item stream) ----
                def qp(c, q0):
                    ps = psA.tile([128, QCH], dt.float32, tag="pA", name="pA")
                    for j in range(4):
                        nc.tensor.matmul(
                            ps,
                            lhsT=wsb["wq"][:, j, c * 128:(c + 1) * 128],
                            rhs=qTin[:, j, q0 * QCH:(q0 + 1) * QCH],
                            start=(j == 0), stop=(j == 3),
                        )
                    nc.vector.tensor_copy(qT[c][:, q0 * QCH:(q0 + 1) * QCH], ps)

                def kp(c, ki):
                    s, w = kcols[ki]
                    if ki == 0 and c > 0:
                        nc.vector.memset(kTz[c], 0.0)
                    ps = psA.tile([128, QCH], dt.float32, tag="pA", name="pA")
                    for j in range(4):
                        nc.tensor.matmul(
                            ps[:, :w],
                            lhsT=wsb["wk"][:, j, c * 128:(c + 1) * 128],
                            rhs=kTin[:, j, s:s + w],
                            start=(j == 0), stop=(j == 3),
                        )
                    nc.vector.tensor_copy(kTz[c][0:64, 0, s:s + w], ps[0:64, :w])
                    nc.vector.tensor_copy(kTz[c][64:128, 1, s:s + w], ps[64:128, :w])

                def vp(t):
                    # one N=512 matmul projects k-tile t for ALL 4 c-chunks
                    if t >= 3:
                        nc.vector.memset(vsb[:, t], 1.0)
                    ps = psA.tile([128, QCH], dt.float32, tag="pA", name="pA")
                    for j in range(4):
                        nc.tensor.matmul(
                            ps,
                            lhsT=vTin[:, j, t * 128:(t + 1) * 128],
                            rhs=wsb["wv"][:, j, :],
                            start=(j == 0), stop=(j == 3),
                        )
                    for c in range(4):
                        for h in range(2):
                            nc.vector.tensor_copy(
                                vsb[:, t, c, h, 0:64],
                                ps[:, c * 128 + h * 64:c * 128 + (h + 1) * 64])
                            nc.vector.tensor_copy(vsb[:, t, c, h, 64:65],
                                                  vind[:, t:t + 1])

                def proj_tasks(c):
                    # vproj is done once (all c) during c=0's stream
                    return ([lambda q0=q0: qp(c, q0) for q0 in range(NQ)]
                            + [lambda ki=ki: kp(c, ki) for ki in range(len(kcols))])

                # ---- attention pipeline, items = (h, q0, group) ----
                items = [(h, q0, gi) for h in range(2) for q0 in range(NQ)
                         for gi in range(len(groups))]
                NG = len(groups)

                def emit_energy(c, h, q0, gi):
                    grp = groups[gi]
                    eh = psE.tile([128, GW], dt.float32, tag="e", name="e")
                    for b, t in enumerate(grp):
                        nc.tensor.matmul(
                            eh[:, b * QCH:(b + 1) * QCH],
                            lhsT=kTz[c][:, h, t * 128:(t + 1) * 128],
                            rhs=qT[c][:, q0 * QCH:(q0 + 1) * QCH],
                            start=True, stop=True,
                        )
                    return eh

                def emit_exp_av(c, h, q0, gi, eh, av):
                    grp = groups[gi]
                    gw = len(grp) * QCH
                    ex = sbx.tile([128, GW], dt.bfloat16, tag="x", name="x")
                    nc.scalar.activation(
                        ex[:, :gw], eh[:, :gw],
                        mybir.ActivationFunctionType.Exp,
                        scale=SCALE,
                    )
                    for b, t in enumerate(grp):
                        nc.tensor.matmul(
                            av,
                            lhsT=vsb[:, t, c, h, :],
                            rhs=ex[:, b * QCH:(b + 1) * QCH],
                            start=(t == 0), stop=(t == T - 1),
                        )

                def emit_out(c, h, q0, av):
                    ot = sbo.tile([65, QCH], dt.float32, tag="ot", name="ot")
                    nc.vector.tensor_copy(ot, av[0:65, :])
                    nc.sync.dma_start(
                        out_d[(c * 2 + h) * 65:(c * 2 + h) * 65 + 65,
                              q0 * QCH:(q0 + 1) * QCH], ot)

                # c=0's own projections carry data deadlines (item index by
                # which they must be emitted); later cs' are paced evenly.
                def c0_deadline(kind, arg):
                    if kind == "q":  # needed by E(0, q0, 0)
                        return arg * NG
                    if kind == "k":  # chunk ki first used by E of the group
                        t_lo = kcols[arg][0] // 128   # holding its first tile
                        return min(gi for gi, grp in enumerate(groups)
                                   if t_lo in grp)
                    # "v": tile t needed by AV(group of t), emitted 1 item late
                    g = next(gi for gi, grp in enumerate(groups) if arg in grp)
                    return min(g + 1, len(items) - 1)

                sched: dict = {i: [] for i in range(len(items))}
                for kind, n_args in (("q", NQ), ("k", len(kcols))):
                    for a in range(n_args):
                        fn = {"q": qp, "k": kp}[kind]
                        sched[c0_deadline(kind, a)].append(
                            lambda a=a, fn=fn: fn(0, a))
                for t in range(T):
                    sched[c0_deadline("v", t)].append(lambda t=t: vp(t))

                av_cur = None

                def flush_prev(prev):
                    nonlocal av_cur
                    pc, ph, pq0, pgi, peh = prev
                    if pgi == 0:
                        av_cur = psO.tile([128, QCH], dt.float32, tag="av",
                                          name="av")
                    emit_exp_av(pc, ph, pq0, pgi, peh, av_cur)
                    if pgi == NG - 1:
                        emit_out(pc, ph, pq0, av_cur)

                prev = None
                for c in range(4):
                    tasks = proj_tasks(c + 1) if c < 3 else []
                    n_it = len(items)
                    s0 = 0
                    span = n_it - s0
                    for i, (h, q0, gi) in enumerate(items):
                        if c == 0:
                            for fn in sched[i]:
                                fn()
                        # evenly paced projections for c+1
                        if i >= s0:
                            p = i - s0
                            for fn in tasks[(p * len(tasks)) // span:
                                            ((p + 1) * len(tasks)) // span]:
                                fn()
                        eh = emit_energy(c, h, q0, gi)
                        if prev is not None:
                            flush_prev(prev)
                        prev = (c, h, q0, gi, eh)
                flush_prev(prev)

    nc.compile()
    return nc


def _prepare(queries, keys, values, mask):
    """Host-side sharding: transpose, compact kv by mask, validity tiles."""
    m = np.asarray(mask).reshape(N, KLEN) != 0
    idx = [np.nonzero(m[n])[0] for n in range(N)]
    cnts = [len(i) for i in idx]
    T = max(1, (max(cnts) + 127) // 128)
    KC = 128 * T

    kT_full = np.ascontiguousarray(np.asarray(keys, np.float32)[0].T)
    vT_full = np.ascontiguousarray(np.asarray(values, np.float32)[0].T)
    q32 = np.asarray(queries, np.float32)

    qT_n, kT_n, vT_n, vind_n = [], [], [], []
    for n in range(N):
        kt = np.zeros((KVDIM, KC), np.float32)
        vt = np.zeros((KVDIM, KC), np.float32)
        kt[:, :cnts[n]] = kT_full[:, idx[n]]
        vt[:, :cnts[n]] = vT_full[:, idx[n]]
        ind = (np.arange(KC) < cnts[n]).astype(np.float32)
        vind_n.append(np.ascontiguousarray(ind.reshape(T, 128).T))
        kT_n.append(kt.astype(BF16))
        vT_n.append(vt.astype(BF16))
        qT_n.append(np.ascontiguousarray(q32[n].T).astype(BF16))
    return T, qT_n, kT_n, vT_n, vind_n


def kernel(queries, keys, values, mask, Wq, Wk, Wv, _trace=False):
    global last_exec_time_ns, last_results
    T, qT_n, kT_n, vT_n, vind_n = _prepare(queries, keys, values, mask)

    w_g = {}
    for nm, W in (("wq", Wq), ("wk", Wk), ("wv", Wv)):
        W = np.asarray(W, np.float32)
        w_g[nm] = [np.ascontiguousarray(W[:, g * 512:(g + 1) * 512]).astype(BF16)
                   for g in range(2)]

    nc = _cache.get(T)
    if nc is None:
        nc = _cache.setdefault(T, _build(T))

    in_maps = []
    for core in range(N_CORES):
        n, g = core // 2, core % 2
        in_maps.append({
            "qt": qT_n[n], "kt": kT_n[n], "vt": vT_n[n],
            "wq": w_g["wq"][g], "wk": w_g["wk"][g], "wv": w_g["wv"][g],
            "vind": vind_n[n],
        })

    res = run_bass_kernel_spmd(nc, in_maps, core_ids=list(range(N_CORES)),
                               trace=bool(_trace))
    last_exec_time_ns = res.exec_time_ns
    last_results = res

    full = np.empty((N, QLEN, EMBED), np.float32)
    for core in range(N_CORES):
        n, g = core // 2, core % 2
        o = res.results[core]["out"].reshape(8, 65, QLEN)
        vals = o[:, :64, :] / o[:, 64:65, :]          # [8, 64, QLEN]
        full[n, :, g * 512:(g + 1) * 512] = (
            vals.transpose(2, 0, 1).reshape(QLEN, 512)
        )
    return full


# revision 13
# speedup vs baseline: 1.1403x; 1.1161x over previous
"""MultiHeadCrossAttention Trainium2 kernel (8 NeuronCores, SPMD).

Sharding: core c -> (n = c // 2, g = c % 2). Each core handles one query
batch n and half the heads (8 of 16, embed slice g*512:(g+1)*512).

Host side: transpose queries/keys/values into [dim, tokens] layout (the
TensorEngine contracts along the partition dim, so both matmul operands
need the contraction dim on partitions), compact keys/values along KLEN
by the per-n mask (~50% survive), pad to KC = 128*T, cast to bf16.
The unnormalized AV outputs and softmax denominators come back per core;
the host divides while assembling/transposing the full output.

Device side per core (all matmuls bf16, fp32 PSUM accumulation):
  - qT/kT projections in transposed layout (lhsT = W chunk, rhs = xT);
    kT lands in per-head zero-padded slots (kTz) so the energy matmuls
    run with K=128 - full PE-array activity keeps the HAM clock at
    2.4 GHz (K=64 matmuls measurably re-throttle the PE to 1.2 GHz).
  - v projection in [k, emb] layout (lhsT = vT k-tile, rhs = W chunk).
  - energyT[k, q] = kTz.T @ qT per head, one PSUM bank per (head, k-tile).
  - exp on ScalarE (scale=1/8) PSUM->SBUF bf16. This is the kernel
    bottleneck: softmax exp runs at 1 elem/cycle/lane at 1.2 GHz and only
    ScalarE can do it, so the whole kernel is paced by the ACTIVATE
    stream. Items are (head, q-chunk, group of 3 k-tiles): one FD=1536
    ACTIVATE per item (96 total) amortizes the ~300-cycle per-ACTIVATE
    overhead that an FD=1024 split pays 160 times.
  - AV with lhsT = [v_h | valid-indicator | filler] (M=128) accumulated
    over k-tiles into one PSUM bank per (head, q-chunk): row 64 of the
    accumulator is the softmax denominator for free. Padded k rows have
    v=0 and indicator=0 so they contribute nothing anywhere.
  - PSUM budget (8 banks): energy 2 bufs x 3 banks + projections 1 +
    AV accumulator 1.
  - software pipeline: energy of item i+1 runs on the PE while exp of
    item i streams on ScalarE, kept alive across q-chunk AND head-pair
    boundaries; projection steps (incl. c=0's own) are injected into the
    item stream with data-deadlines so the first exp fires as soon as
    the first weight/q/k DMA chunks land (~10us instead of ~34us);
    inputs stream in column-chunks ordered by first use; junk matmuls
    during the initial DMA window pre-warm the PE clock gate; a dummy
    ACTIVATE preloads the exp table set (~2.7us) off the critical path.
"""

import math
import sys
from contextlib import ExitStack

import numpy as np

for _p in ("/opt/trn_rl_repo",):
    if _p not in sys.path:
        sys.path.insert(0, _p)

import ml_dtypes

import concourse.bass as bass  # noqa: F401  (import registers lowering deps)
import concourse.tile as tile
from concourse import bacc, mybir
from concourse.bass_utils import run_bass_kernel_spmd

BF16 = ml_dtypes.bfloat16

N, QLEN, KLEN = 4, 2048, 2048
QDIM = KVDIM = 512
EMBED, HEADS = 1024, 16
HEAD_DIM = 64
N_CORES = 8
QCH = 512  # q-chunk width (one PSUM bank of fp32)
SCALE = 1.0 / math.sqrt(HEAD_DIM)

_cache: dict = {}
last_exec_time_ns = None
last_results = None


def _build(T: int, ql: int = QLEN):
    """Build the per-core Bass program for KC = 128*T compacted kv tokens."""
    KC = 128 * T
    dt = mybir.dt
    nc = bacc.Bacc("TRN2", target_bir_lowering=False, debug=False)

    qT_d = nc.dram_tensor("qt", [QDIM, ql], dt.bfloat16, kind="ExternalInput").ap()
    kT_d = nc.dram_tensor("kt", [KVDIM, KC], dt.bfloat16, kind="ExternalInput").ap()
    vT_d = nc.dram_tensor("vt", [KVDIM, KC], dt.bfloat16, kind="ExternalInput").ap()
    wq_d = nc.dram_tensor("wq", [QDIM, 512], dt.bfloat16, kind="ExternalInput").ap()
    wk_d = nc.dram_tensor("wk", [KVDIM, 512], dt.bfloat16, kind="ExternalInput").ap()
    wv_d = nc.dram_tensor("wv", [KVDIM, 512], dt.bfloat16, kind="ExternalInput").ap()
    # per-row validity indicator (1.0 real kv token, 0.0 pad), repeated
    # 8x per tile so all (c, h) slots fill with ONE copy: [128, T*8]
    vind_d = nc.dram_tensor("vind", [128, T * 8], dt.float32, kind="ExternalInput").ap()
    # rows (c*2+h)*65 .. +64: unnormalized AV.T ; row +64: denominator
    out_d = nc.dram_tensor("out", [520, ql], dt.float32, kind="ExternalOutput").ap()

    # [p, j, col] views: one DMA instruction loads all 4 j-tiles of a
    # column chunk (DMA issue cost is per instruction, ~0.6us each on the
    # issuing queue - instruction count, not bytes, gates the startup)
    qT_r = qT_d.rearrange("(j p) c -> p j c", j=4, p=128)
    kT_r = kT_d.rearrange("(j p) c -> p j c", j=4, p=128)
    vT_r = vT_d.rearrange("(j p) c -> p j c", j=4, p=128)
    w_r = {"wq": wq_d.rearrange("(j p) c -> p j c", j=4, p=128),
           "wk": wk_d.rearrange("(j p) c -> p j c", j=4, p=128),
           "wv": wv_d.rearrange("(j p) c -> p j c", j=4, p=128)}

    NQ = ql // QCH
    kcols = [(s, min(512, KC - s)) for s in range(0, KC, 512)]
    # k-tile groups of 3 for batched exp (psE tile = 3 banks, FD=1536)
    groups = [tuple(range(t, min(t + 3, T))) for t in range(0, T, 3)]
    GW = 512 * min(3, T)

    with tile.TileContext(nc) as tc:
        with ExitStack() as ctx:
            persist = ctx.enter_context(tc.tile_pool(name="persist", bufs=1))

            qTin = persist.tile([128, 4, ql], dt.bfloat16, tag="qTin", name="qTin")
            kTin = persist.tile([128, 4, KC], dt.bfloat16, tag="kTin", name="kTin")
            vTin = persist.tile([128, 4, KC], dt.bfloat16, tag="vTin", name="vTin")
            wsb = {nm: persist.tile([128, 4, 512], dt.bfloat16, tag=nm, name=nm)
                   for nm in ("wq", "wk", "wv")}
            qT = [persist.tile([128, ql], dt.bfloat16, tag=f"qT{c}", name=f"qT{c}") for c in range(4)]
            # kTz[c][:, h, :]: rows h*64..h*64+63 hold head h's kT rows, the
            # other 64 rows stay zero -> energy matmuls run with K=128 (full
            # PE array activity) at the same stream cost.
            kTz = [persist.tile([128, 2, KC], dt.bfloat16, tag=f"kTz{c}", name=f"kTz{c}") for c in range(4)]
            # [v_h (64) | indicator (1) | filler (63)]: M=128 keeps the full
            # array busy; output rows 65-127 are ignored.
            vsb = persist.tile([128, T, 4, 2, 128], dt.bfloat16, tag="v", name="v")
            vind = persist.tile([128, T * 8], dt.float32, tag="vind", name="vind")
            junk = persist.tile([128, 512], dt.bfloat16, tag="junk", name="junk")
            dum = persist.tile([128, 1], dt.float32, tag="dum", name="dum")
            dumo = persist.tile([128, 1], dt.float32, tag="dumo", name="dumo")

            # ---- init: preload the exp table set + early memsets for the
            # tiles the first items touch; the rest are deferred into the
            # projection closures so they don't head-of-line block the DVE.
            nc.vector.memset(dum, 0.0)
            nc.scalar.activation(dumo, dum, mybir.ActivationFunctionType.Exp,
                                 scale=1.0)
            nc.vector.memset(junk, 1.0)
            nc.vector.memset(kTz[0], 0.0)
            nc.vector.memset(vsb[:, 0:min(3, T)], 1.0)

            # ---- input DMA, whole [128, X] row-contiguous tiles (DMA
            # transfers are descriptor-rate bound: column-chunked loads
            # have 4x the descriptors and move SLOWER). Ordered by first
            # use; per-j arrival lets the j-accumulation MMs start early.
            for j in range(4):
                nc.sync.dma_start(wsb["wq"][:, j], w_r["wq"][:, j])
            for j in range(4):
                nc.sync.dma_start(wsb["wk"][:, j], w_r["wk"][:, j])
            nc.sync.dma_start(vind, vind_d)
            for j in range(4):
                nc.sync.dma_start(qTin[:, j], qT_r[:, j])
            for j in range(4):
                nc.sync.dma_start(kTin[:, j], kT_r[:, j])
            for j in range(4):
                nc.sync.dma_start(wsb["wv"][:, j], w_r["wv"][:, j])
            for j in range(4):
                nc.sync.dma_start(vTin[:, j], vT_r[:, j])

            with tc.tile_pool(name="psA", bufs=1, space="PSUM") as psA, \
                 tc.tile_pool(name="psE", bufs=2, space="PSUM") as psE, \
                 tc.tile_pool(name="psO", bufs=1, space="PSUM") as psO, \
                 tc.tile_pool(name="sbx", bufs=9) as sbx, \
                 tc.tile_pool(name="sbo", bufs=3) as sbo:

                # PE warm-up during the DMA window (HAM un-throttle)
                ps = psA.tile([128, QCH], dt.float32, tag="pA", name="pA")
                for r in range(6):
                    nc.tensor.matmul(ps, lhsT=junk[:, :128], rhs=junk,
                                     start=(r == 0), stop=(r == 5))

                # ---- projection steps (closures, injected into the
                #      attention item stream) ----
                def qp(c, q0):
                    ps = psA.tile([128, QCH], dt.float32, tag="pA", name="pA")
                    for j in range(4):
                        nc.tensor.matmul(
                            ps,
                            lhsT=wsb["wq"][:, j, c * 128:(c + 1) * 128],
                            rhs=qTin[:, j, q0 * QCH:(q0 + 1) * QCH],
                            start=(j == 0), stop=(j == 3),
                        )
                    nc.vector.tensor_copy(qT[c][:, q0 * QCH:(q0 + 1) * QCH], ps)

                def kp(c, ki):
                    s, w = kcols[ki]
                    if ki == 0 and c > 0:
                        nc.vector.memset(kTz[c], 0.0)
                    ps = psA.tile([128, QCH], dt.float32, tag="pA", name="pA")
                    for j in range(4):
                        nc.tensor.matmul(
                            ps[:, :w],
                            lhsT=wsb["wk"][:, j, c * 128:(c + 1) * 128],
                            rhs=kTin[:, j, s:s + w],
                            start=(j == 0), stop=(j == 3),
                        )
                    nc.vector.tensor_copy(kTz[c][0:64, 0, s:s + w], ps[0:64, :w])
                    nc.vector.tensor_copy(kTz[c][64:128, 1, s:s + w], ps[64:128, :w])

                def vp(t):
                    # one N=512 matmul projects k-tile t for ALL 4 c-chunks
                    if t >= 3:
                        nc.vector.memset(vsb[:, t], 1.0)
                    ps = psA.tile([128, QCH], dt.float32, tag="pA", name="pA")
                    for j in range(4):
                        nc.tensor.matmul(
                            ps,
                            lhsT=vTin[:, j, t * 128:(t + 1) * 128],
                            rhs=wsb["wv"][:, j, :],
                            start=(j == 0), stop=(j == 3),
                        )
                    nc.vector.tensor_copy(
                        vsb[:, t, :, :, 0:64],
                        ps.rearrange("p (c h e) -> p c h e", c=4, h=2, e=64))
                    nc.vector.tensor_copy(
                        vsb[:, t, :, :, 64:65],
                        vind[:, t * 8:(t + 1) * 8].rearrange(
                            "p (c h o) -> p c h o", c=4, h=2, o=1))

                def proj_tasks(c):
                    # vproj is done once (all c) during c=0's stream
                    return ([lambda q0=q0: qp(c, q0) for q0 in range(NQ)]
                            + [lambda ki=ki: kp(c, ki) for ki in range(len(kcols))])

                # ---- attention pipeline, items = (h, q0, group) ----
                items = [(h, q0, gi) for h in range(2) for q0 in range(NQ)
                         for gi in range(len(groups))]
                NG = len(groups)

                def emit_energy(c, h, q0, gi):
                    grp = groups[gi]
                    eh = psE.tile([128, GW], dt.float32, tag="e", name="e")
                    for b, t in enumerate(grp):
                        nc.tensor.matmul(
                            eh[:, b * QCH:(b + 1) * QCH],
                            lhsT=kTz[c][:, h, t * 128:(t + 1) * 128],
                            rhs=qT[c][:, q0 * QCH:(q0 + 1) * QCH],
                            start=True, stop=True,
                        )
                    return eh

                def emit_exp_av(c, h, q0, gi, eh, av):
                    grp = groups[gi]
                    gw = len(grp) * QCH
                    ex = sbx.tile([128, GW], dt.bfloat16, tag="x", name="x")
                    nc.scalar.activation(
                        ex[:, :gw], eh[:, :gw],
                        mybir.ActivationFunctionType.Exp,
                        scale=SCALE,
                    )
                    for b, t in enumerate(grp):
                        nc.tensor.matmul(
                            av,
                            lhsT=vsb[:, t, c, h, :],
                            rhs=ex[:, b * QCH:(b + 1) * QCH],
                            start=(t == 0), stop=(t == T - 1),
                        )

                def emit_out(c, h, q0, av):
                    ot = sbo.tile([65, QCH], dt.float32, tag="ot", name="ot")
                    nc.vector.tensor_copy(ot, av[0:65, :])
                    nc.sync.dma_start(
                        out_d[(c * 2 + h) * 65:(c * 2 + h) * 65 + 65,
                              q0 * QCH:(q0 + 1) * QCH], ot)

                # c=0's own projections carry data deadlines (item index by
                # which they must be emitted); later cs' are paced evenly.
                def c0_deadline(kind, arg):
                    if kind == "q":  # needed by E(0, q0, 0)
                        return arg * NG
                    if kind == "k":  # chunk ki first used by E of the group
                        t_lo = kcols[arg][0] // 128   # holding its first tile
                        return min(gi for gi, grp in enumerate(groups)
                                   if t_lo in grp)
                    # "v": tile t needed by AV(group of t), emitted 1 item late
                    g = next(gi for gi, grp in enumerate(groups) if arg in grp)
                    return min(g + 1, len(items) - 1)

                sched: dict = {i: [] for i in range(len(items))}
                for kind, n_args in (("q", NQ), ("k", len(kcols))):
                    for a in range(n_args):
                        fn = {"q": qp, "k": kp}[kind]
                        sched[c0_deadline(kind, a)].append(
                            lambda a=a, fn=fn: fn(0, a))
                for t in range(T):
                    sched[c0_deadline("v", t)].append(lambda t=t: vp(t))

                av_cur = None
                av_queue = []

                def emit_exp(c, h, q0, gi, eh):
                    grp = groups[gi]
                    gw = len(grp) * QCH
                    ex = sbx.tile([128, GW], dt.bfloat16, tag="x", name="x")
                    nc.scalar.activation(
                        ex[:, :gw], eh[:, :gw],
                        mybir.ActivationFunctionType.Exp,
                        scale=SCALE,
                    )
                    return ex

                def emit_av(c, h, q0, gi, ex):
                    nonlocal av_cur
                    if gi == 0:
                        av_cur = psO.tile([128, QCH], dt.float32, tag="av",
                                          name="av")
                    for b, t in enumerate(groups[gi]):
                        nc.tensor.matmul(
                            av_cur,
                            lhsT=vsb[:, t, c, h, :],
                            rhs=ex[:, b * QCH:(b + 1) * QCH],
                            start=(t == 0), stop=(t == T - 1),
                        )
                    if gi == NG - 1:
                        emit_out(c, h, q0, av_cur)

                # AV matmuls lag the E/exp stream by up to 6 items during
                # c=0: they depend on vTin/vproj, which land LAST in the
                # input DMA - an in-order stall on AV would block every
                # later energy matmul and starve ScalarE.
                prev = None
                k = 0
                for c in range(4):
                    tasks = proj_tasks(c + 1) if c < 3 else []
                    n_it = len(items)
                    for i, (h, q0, gi) in enumerate(items):
                        if c == 0:
                            for fn in sched[i]:
                                fn()
                        for fn in tasks[(i * len(tasks)) // n_it:
                                        ((i + 1) * len(tasks)) // n_it]:
                            fn()
                        eh = emit_energy(c, h, q0, gi)
                        if prev is not None:
                            pc, ph, pq0, pgi, peh = prev
                            ex = emit_exp(pc, ph, pq0, pgi, peh)
                            av_queue.append((pc, ph, pq0, pgi, ex))
                        target = 6 if k < 24 else max(0, 6 - (k - 24))
                        while len(av_queue) > target:
                            emit_av(*av_queue.pop(0))
                        prev = (c, h, q0, gi, eh)
                        k += 1
                pc, ph, pq0, pgi, peh = prev
                ex = emit_exp(pc, ph, pq0, pgi, peh)
                av_queue.append((pc, ph, pq0, pgi, ex))
                for entry in av_queue:
                    emit_av(*entry)

    nc.compile()
    return nc


def _prepare(queries, keys, values, mask):
    """Host-side sharding: transpose, compact kv by mask, validity tiles."""
    m = np.asarray(mask).reshape(N, KLEN) != 0
    idx = [np.nonzero(m[n])[0] for n in range(N)]
    cnts = [len(i) for i in idx]
    T = max(1, (max(cnts) + 127) // 128)
    KC = 128 * T

    kT_full = np.ascontiguousarray(np.asarray(keys, np.float32)[0].T)
    vT_full = np.ascontiguousarray(np.asarray(values, np.float32)[0].T)
    q32 = np.asarray(queries, np.float32)

    qT_n, kT_n, vT_n, vind_n = [], [], [], []
    for n in range(N):
        kt = np.zeros((KVDIM, KC), np.float32)
        vt = np.zeros((KVDIM, KC), np.float32)
        kt[:, :cnts[n]] = kT_full[:, idx[n]]
        vt[:, :cnts[n]] = vT_full[:, idx[n]]
        ind = (np.arange(KC) < cnts[n]).astype(np.float32)
        vind_n.append(np.ascontiguousarray(
            np.repeat(ind.reshape(T, 128).T, 8, axis=1)))
        kT_n.append(kt.astype(BF16))
        vT_n.append(vt.astype(BF16))
        qT_n.append(np.ascontiguousarray(q32[n].T).astype(BF16))
    return T, qT_n, kT_n, vT_n, vind_n


def kernel(queries, keys, values, mask, Wq, Wk, Wv, _trace=False):
    global last_exec_time_ns, last_results
    T, qT_n, kT_n, vT_n, vind_n = _prepare(queries, keys, values, mask)

    w_g = {}
    for nm, W in (("wq", Wq), ("wk", Wk), ("wv", Wv)):
        W = np.asarray(W, np.float32)
        w_g[nm] = [np.ascontiguousarray(W[:, g * 512:(g + 1) * 512]).astype(BF16)
                   for g in range(2)]

    nc = _cache.get(T)
    if nc is None:
        nc = _cache.setdefault(T, _build(T))

    in_maps = []
    for core in range(N_CORES):
        n, g = core // 2, core % 2
        in_maps.append({
            "qt": qT_n[n], "kt": kT_n[n], "vt": vT_n[n],
            "wq": w_g["wq"][g], "wk": w_g["wk"][g], "wv": w_g["wv"][g],
            "vind": vind_n[n],
        })

    res = run_bass_kernel_spmd(nc, in_maps, core_ids=list(range(N_CORES)),
                               trace=bool(_trace))
    last_exec_time_ns = res.exec_time_ns
    last_results = res

    full = np.empty((N, QLEN, EMBED), np.float32)
    for core in range(N_CORES):
        n, g = core // 2, core % 2
        o = res.results[core]["out"].reshape(8, 65, QLEN)
        vals = o[:, :64, :] / o[:, 64:65, :]          # [8, 64, QLEN]
        full[n, :, g * 512:(g + 1) * 512] = (
            vals.transpose(2, 0, 1).reshape(QLEN, 512)
        )
    return full
